# revision 20
# baseline (speedup 1.0000x reference)
"""CTREmbedding Trainium2 kernel.

out[b,l,m,e] = interval-embedding interpolation:
    v  = (l < traj_length[b])                       in {0,1}
    ds = v ? mat2[traj_location[b,l]-1, m] : 0
    dt = vector[b,l]
    out = ds * S1[e] + C0[e] + v*Cv[e] + dt*Ct[e] + v*dt*Cvt[e]

Per (b,l) pair the [M,E] block is one K=20 fp32 matmul:
    lhsT [20,128] : rows 0..15 = dsT (dsT[j,p] = ds[16p+j], gathered from a
                    host-rearranged mat2R table; invalid pairs hit a zero row)
                    rows 16..19 = per-pair scalars (1, v, dt, v*dt) bcast x128
    rhs  [20,800] : rows 0..15 = block-diag S1 (rhs[j, j*50+e] = S1[e])
                    rows 16..19 = C-basis vectors tiled 16x along free dim
    out  [128,800]: out[p, j*50+e] = value at m=16p+j  -> partition p owns
                    m in [16p,16p+16) = 3200 contiguous output bytes.

Sharding: 400 (b,l) pairs, 50 per core on 8 cores; each core writes a
contiguous [50, M*E] slice. Per-core roofline = 20.5MB HBM write ~ 57us;
the kernel is output-DMA bound with staged group sizes to fill the pipe.
"""

import numpy as np

B, L, M, E, NLOC = 4, 100, 2048, 50, 4096
EX_SU, EX_SL, EX_TU, EX_TL = 1000.0, 0.0, 86400.0, 0.0

N_CORES = 8
PAIRS = B * L                      # 400
PPC = PAIRS // N_CORES             # 50 pairs per core
JJ = 16                            # m-values per partition
PCH = M // JJ                      # 128 partitions
FREE = JJ * E                      # 800 floats per partition per pair
GROUP = 4                          # pairs per output DMA (4 * 400KB)
SIZES = [1, 3, 2] + [GROUP] * ((PPC - 6) // GROUP)   # leading small groups
LW = PPC * PCH                     # 6400: gather+sigma region width
CW = LW + FREE                     # 7200: + rhs table columns
CW2 = CW + PPC                     # 7250: + idx columns (int32 bits as f32)

_cache = {}


def _build_bass():
    import concourse.bass as bass
    import concourse.tile as tile
    from concourse import bacc, mybir
    from concourse.tile import add_dep_helper

    f32 = mybir.dt.float32
    i32 = mybir.dt.int32

    nc = bacc.Bacc("TRN2", target_bir_lowering=False, debug=False,
                   num_devices=N_CORES)
    mat2r = nc.declare_dram_parameter("mat2r", [(NLOC + 1) * JJ, PCH], f32,
                                      isOutput=False)
    # consts[16:20, 0:6400] = sigma; consts[:, 6400:7200] = rhs table;
    # consts[0:16, 7200:7250] = gather row indices (int32 bits as f32)
    consts = nc.declare_dram_parameter("consts", [20, CW2], f32,
                                       isOutput=False)
    out = nc.declare_dram_parameter("out", [PPC, M * E], f32, isOutput=True)

    with tile.TileContext(nc) as tc:
        with (
            tc.tile_pool(name="const", bufs=1) as cpool,
            tc.tile_pool(name="outp", bufs=3) as opool,
            tc.tile_pool(name="psum", bufs=4, space="PSUM") as ppool,
        ):
            lhs_sb = cpool.tile([20, CW2], f32)
            # rect1 (rhs+idx cols, small, first: gathers wait only on this);
            # rect2 (sigma rows). Gather region [0:16, 0:LW] stays
            # DMA-untouched -- gathers fully overwrite their blocks.
            dma_r1 = nc.sync.dma_start(out=lhs_sb[:, LW:CW2],
                                       in_=consts[:, LW:CW2])
            dma_sig = nc.sync.dma_start(out=lhs_sb[16:20, 0:LW],
                                        in_=consts[16:20, 0:LW])
            # warmup matmul absorbs rect1+(via dep)rect2 waits on PE and
            # starts the HAM ramp before the first real pair arrives
            wps = ppool.tile([PCH, FREE], f32, tag="ps")
            wmm = nc.tensor.matmul(
                out=wps[0:4, 0:4], lhsT=lhs_sb[0:20, CW - 8 : CW - 4],
                rhs=lhs_sb[0:20, CW - 4 : CW], start=True, stop=True,
            )
            add_dep_helper(wmm.ins, dma_sig.ins, True, "absorb sigma wait")

            i0 = 0
            for g, ng in enumerate(SIZES):
                out_sb = opool.tile([PCH, GROUP * FREE], f32, tag="out_sb")
                for q in range(ng):
                    i = i0 + q
                    # per-pair gather: 16 rows (512B each) of mat2r
                    nc.gpsimd.indirect_dma_start(
                        out=lhs_sb[0:JJ, i * PCH : (i + 1) * PCH],
                        out_offset=None,
                        in_=mat2r[:, :],
                        in_offset=bass.IndirectOffsetOnAxis(
                            ap=lhs_sb[0:JJ, CW + i : CW + i + 1].bitcast(i32),
                            axis=0,
                        ),
                    )
                    lhsT = lhs_sb[0:20, i * PCH : (i + 1) * PCH]
                    ps = ppool.tile([PCH, FREE], f32, tag="ps")
                    nc.tensor.matmul(
                        out=ps[:, 0:512], lhsT=lhsT,
                        rhs=lhs_sb[0:20, LW : LW + 512],
                        start=True, stop=True,
                    )
                    nc.tensor.matmul(
                        out=ps[:, 512:FREE], lhsT=lhsT,
                        rhs=lhs_sb[0:20, LW + 512 : CW],
                        start=True, stop=True,
                    )
                    dst = out_sb[:, q * FREE : (q + 1) * FREE]
                    nc.vector.tensor_copy(out=dst[:, 0:512], in_=ps[:, 0:512])
                    nc.scalar.copy(out=dst[:, 512:FREE], in_=ps[:, 512:FREE])
                # DRAM [ng, M*E] block: iteration (p, q, 800-float runs)
                dram_ap = out[i0 : i0 + ng, :].rearrange(
                    "q (p r) -> p q r", p=PCH
                )
                sb_ap = out_sb[:, 0 : ng * FREE].rearrange(
                    "p (q r) -> p q r", q=ng
                )
                nc.sync.dma_start(out=dram_ap, in_=sb_ap)
                i0 += ng
    nc.compile()
    return nc


def kernel(**inputs):
    from concourse.bass_utils import run_bass_kernel_spmd

    traj_location = np.asarray(inputs["traj_location"]).astype(np.int64)
    mat2 = np.asarray(inputs["mat2"], dtype=np.float32)
    vector = np.asarray(inputs["vector"], dtype=np.float32)
    traj_length = np.asarray(inputs["traj_length"]).astype(np.int64)
    emb_su = np.asarray(inputs["emb_su"], dtype=np.float32)
    emb_sl = np.asarray(inputs["emb_sl"], dtype=np.float32)
    emb_tu = np.asarray(inputs["emb_tu"], dtype=np.float32)
    emb_tl = np.asarray(inputs["emb_tl"], dtype=np.float32)

    # ---- host prep: O(B*L) scalars + one mat2 layout rearrange ----
    valid = (np.arange(L)[None, :] < traj_length[:, None]).reshape(-1)  # [400]
    v = valid.astype(np.float32)
    dt = vector.reshape(-1)
    loc0 = (traj_location.reshape(-1) - 1).astype(np.int64)
    gidx = np.where(valid, loc0, NLOC).astype(np.int64)     # NLOC = zero row

    # mat2R[r, j, p] = mat2[r, 16p + j]; row NLOC = zeros
    mat2r = np.ascontiguousarray(np.concatenate(
        [mat2.reshape(NLOC, PCH, JJ).transpose(0, 2, 1),
         np.zeros((1, JJ, PCH), np.float32)], axis=0,
    ).reshape((NLOC + 1) * JJ, PCH))

    # rhs table [20, 800]
    S1 = (emb_su[1] - emb_sl[1]) / (EX_SU - EX_SL)
    C0 = emb_sl[0] + emb_tl[0]
    Cv = (emb_sl[1] + emb_tl[1]) - (emb_sl[0] + emb_tl[0])
    Ct = (emb_tu[0] - emb_tl[0]) / (EX_TU - EX_TL)
    Cvt = ((emb_tu[1] - emb_tl[1]) - (emb_tu[0] - emb_tl[0])) / (EX_TU - EX_TL)
    rhstab = np.zeros((20, FREE), np.float32)
    for j in range(JJ):
        rhstab[j, j * E : (j + 1) * E] = S1
    rhstab[16, :] = np.tile(C0, JJ)
    rhstab[17, :] = np.tile(Cv, JJ)
    rhstab[18, :] = np.tile(Ct, JJ)
    rhstab[19, :] = np.tile(Cvt, JJ)

    in_maps = []
    for c in range(N_CORES):
        sl = slice(c * PPC, (c + 1) * PPC)
        gc, vc, dtc = gidx[sl], v[sl], dt[sl]
        idx = (gc[None, :] * JJ + np.arange(JJ)[:, None]).astype(np.int32)
        sig = np.stack([np.ones(PPC, np.float32), vc, dtc, vc * dtc])
        sigma = np.repeat(sig, PCH, axis=1).astype(np.float32)
        consts = np.zeros((20, CW2), np.float32)
        consts[16:20, 0:LW] = sigma
        consts[:, LW:CW] = rhstab
        consts[0:JJ, CW:CW2] = idx.view(np.float32)
        in_maps.append({"mat2r": mat2r, "consts": consts})

    if "nc" not in _cache:
        _cache["nc"] = _build_bass()
    res = run_bass_kernel_spmd(_cache["nc"], in_maps,
                               core_ids=list(range(N_CORES)))
    parts = [res.results[c]["out"].reshape(PPC, M, E) for c in range(N_CORES)]
    return np.concatenate(parts, axis=0).reshape(B, L, M, E).astype(np.float32)
